# revision 3
# baseline (speedup 1.0000x reference)
"""NMS-detection network on 8 Trainium2 NeuronCores.

Sharding: (batch=4) x (H-half=2) -> 8 shards. Each shard runs the small conv
stack + instance norm on the full image of its batch entry (exact global
statistics without any cross-core collective), then computes the heavy
windowed-NMS/softmax stage only for its own 256-row half (plus a 7-row halo
for the 15x15 windows). One pmap call drives all 8 cores; the host just
reassembles the halves.
"""
import os

os.environ.setdefault("NEURON_CC_FLAGS", "--auto-cast=none")

import numpy as np
import jax
import jax.numpy as jnp

try:
    jax.config.update("jax_compilation_cache_dir", "/tmp/jax_cache")
except Exception:
    pass
try:
    jax.config.update("jax_default_matmul_precision", "highest")
except Exception:
    pass

EPS = 1e-8
NMS_K = 15
COM_NMS = 7.0
COM_BETA = 100.0  # score and scale softmax strengths are both 100 -> p1 == p2

B, H, W, C, S = 4, 512, 512, 16, 10
HALF = H // 2
NMS_HALO = 7
NMSROWS = HALF + 2 * NMS_HALO  # 270? no: 256 + 14 = 270 -> but clipped slice is 263
SLICE_ROWS = HALF + 2 * NMS_HALO  # 270: chained 15-row windows need a 14-row halo


def _conv(x, w, b=None, pad=1, groups=1):
    y = jax.lax.conv_general_dilated(
        x, w, (1, 1), [(pad, pad), (pad, pad)],
        dimension_numbers=('NCHW', 'OIHW', 'NCHW'),
        feature_group_count=groups)
    if b is not None:
        y = y + b[None, :, None, None]
    return y


def _bn(x, s, b):
    return x * s[None, :, None, None] + b[None, :, None, None]


def _inv_res(x, dw_w, bna_s, bna_b, pw_w, bnb_s, bnb_b):
    h = _conv(x, dw_w, pad=1, groups=x.shape[1])
    h = jnp.clip(_bn(h, bna_s, bna_b), 0.0, 6.0)
    h = _conv(h, pw_w, pad=0)
    h = _bn(h, bnb_s, bnb_b)
    return x + h


def _pool_h_then_w(x, init, op):
    p = NMS_K // 2
    x = jax.lax.reduce_window(x, init, op, (1, 1, NMS_K, 1), (1, 1, 1, 1),
                              [(0, 0), (0, 0), (p, p), (0, 0)])
    x = jax.lax.reduce_window(x, init, op, (1, 1, 1, NMS_K), (1, 1, 1, 1),
                              [(0, 0), (0, 0), (0, 0), (p, p)])
    return x


def _shard_fn(photo, off_nms, off_out,
              w0, b0, dw1_w, bn1a_s, bn1a_b, pw1_w, bn1b_s, bn1b_b,
              dw2_w, bn2a_s, bn2a_b, pw2_w, bn2b_s, bn2b_b, ws, bs, scale_list):
    # photo: (1, 1, H, W) -- full image; conv stack + exact instance norm
    x = _conv(photo, w0, b0)
    x = _inv_res(x, dw1_w, bn1a_s, bn1a_b, pw1_w, bn1b_s, bn1b_b)
    x = _inv_res(x, dw2_w, bn2a_s, bn2a_b, pw2_w, bn2b_s, bn2b_b)
    s = _conv(x, ws, bs)                                   # (1, S, H, W)
    mu = s.mean(axis=(2, 3), keepdims=True)
    var = s.var(axis=(2, 3), keepdims=True)
    y = (s - mu) * jax.lax.rsqrt(var + 1e-5)
    y = jax.nn.leaky_relu(y, negative_slope=0.01)

    # NMS stage on own half only: rows [off_nms, off_nms + 263)
    yn = jax.lax.dynamic_slice(y, (0, 0, off_nms, 0), (1, S, SLICE_ROWS, W))
    mc = yn.max(axis=1, keepdims=True)
    m = _pool_h_then_w(mc, -jnp.inf, jax.lax.max)
    e = jnp.exp(COM_NMS * (yn - m))
    se = _pool_h_then_w(e.sum(axis=1, keepdims=True), 0.0, jax.lax.add)
    probs = e / (se + EPS)

    mx = probs.max(axis=1, keepdims=True)
    e1 = jnp.exp(COM_BETA * (probs - mx))
    p1 = e1 / (e1.sum(axis=1, keepdims=True) + EPS)
    score = (probs * p1).sum(axis=1)                       # (1, 263, W)
    scale = (scale_list[None, :, None, None] * p1).sum(axis=1)

    sc = jax.lax.dynamic_slice(score, (0, off_out, 0), (1, HALF, W))[0]
    sl = jax.lax.dynamic_slice(scale, (0, off_out, 0), (1, HALF, W))[0]
    return sc, sl


_PFN = None


def _get_pfn():
    global _PFN
    if _PFN is None:
        _PFN = jax.pmap(_shard_fn)
    return _PFN


def _run_sharded(inputs):
    photos = np.asarray(inputs['photos'], np.float32)
    slabs = np.empty((8, 1, 1, H, W), np.float32)
    off_nms = np.empty((8,), np.int32)
    off_out = np.empty((8,), np.int32)
    for i in range(8):
        b, half = i // 2, i % 2
        slabs[i, 0] = photos[b]
        if half == 0:
            off_nms[i], off_out[i] = 0, 0           # rows [0,263) -> out [0,256)
        else:
            off_nms[i], off_out[i] = H - SLICE_ROWS, 2 * NMS_HALO  # [242,512) -> [256,512)

    names = ['w0', 'b0', 'dw1_w', 'bn1a_s', 'bn1a_b', 'pw1_w', 'bn1b_s',
             'bn1b_b', 'dw2_w', 'bn2a_s', 'bn2a_b', 'pw2_w', 'bn2b_s',
             'bn2b_b', 'ws', 'bs', 'scale_list']
    reps = [np.broadcast_to(np.asarray(inputs[k], np.float32),
                            (8,) + np.asarray(inputs[k]).shape) for k in names]
    sc, sl = _get_pfn()(slabs, off_nms, off_out, *reps)
    sc = np.asarray(sc).reshape(B, H, W)[..., None]
    sl = np.asarray(sl).reshape(B, H, W)[..., None]
    return sc.astype(np.float32), sl.astype(np.float32)


def _run_cpu(inputs):
    cpu = jax.devices('cpu')[0]
    fin = {k: jax.device_put(np.asarray(v), cpu) for k, v in inputs.items()}

    def full(photos, w0, b0, dw1_w, bn1a_s, bn1a_b, pw1_w, bn1b_s, bn1b_b,
             dw2_w, bn2a_s, bn2a_b, pw2_w, bn2b_s, bn2b_b, ws, bs, scale_list):
        x = _conv(photos, w0, b0)
        x = _inv_res(x, dw1_w, bn1a_s, bn1a_b, pw1_w, bn1b_s, bn1b_b)
        x = _inv_res(x, dw2_w, bn2a_s, bn2a_b, pw2_w, bn2b_s, bn2b_b)
        s = _conv(x, ws, bs)
        mu = s.mean(axis=(2, 3), keepdims=True)
        var = s.var(axis=(2, 3), keepdims=True)
        y = (s - mu) * jax.lax.rsqrt(var + 1e-5)
        y = jax.nn.leaky_relu(y, negative_slope=0.01)
        mc = y.max(axis=1, keepdims=True)
        m = _pool_h_then_w(mc, -jnp.inf, jax.lax.max)
        e = jnp.exp(COM_NMS * (y - m))
        se = _pool_h_then_w(e.sum(axis=1, keepdims=True), 0.0, jax.lax.add)
        probs = e / (se + EPS)
        mx = probs.max(axis=1, keepdims=True)
        e1 = jnp.exp(COM_BETA * (probs - mx))
        p1 = e1 / (e1.sum(axis=1, keepdims=True) + EPS)
        score = (probs * p1).sum(axis=1, keepdims=True)
        scale = (scale_list[None, :, None, None] * p1).sum(axis=1, keepdims=True)
        return score.transpose(0, 2, 3, 1), scale.transpose(0, 2, 3, 1)

    sc, sl = jax.jit(full, device=cpu)(**fin)
    return np.asarray(sc), np.asarray(sl)


def kernel(**inputs):
    try:
        return _run_sharded(inputs)
    except Exception as ex:  # fall back to exact single-device compute
        import traceback
        traceback.print_exc()
        print(f"[kernel] sharded path failed ({ex!r}); using CPU fallback",
              flush=True)
        return _run_cpu(inputs)


# revision 4
# speedup vs baseline: 1.3617x; 1.3617x over previous
"""NMS-detection network on 8 Trainium2 NeuronCores.

Sharding: (batch=4) x (H-half=2) -> 8 shards. Each shard runs the small conv
stack + instance norm on the full image of its batch entry (exact global
statistics without any cross-core collective), then computes the heavy
windowed-NMS/softmax stage only for its own 256-row half (plus a 7-row halo
for the 15x15 windows). One pmap call drives all 8 cores; the host just
reassembles the halves.
"""
import os

os.environ.setdefault("NEURON_CC_FLAGS", "--auto-cast=none")

import numpy as np
import jax
import jax.numpy as jnp

try:
    jax.config.update("jax_compilation_cache_dir", "/tmp/jax_cache")
except Exception:
    pass
try:
    jax.config.update("jax_default_matmul_precision", "highest")
except Exception:
    pass

EPS = 1e-8
NMS_K = 15
COM_NMS = 7.0
COM_BETA = 100.0  # score and scale softmax strengths are both 100 -> p1 == p2

B, H, W, C, S = 4, 512, 512, 16, 10
HALF = H // 2
NMS_HALO = 7
NMSROWS = HALF + 2 * NMS_HALO  # 270? no: 256 + 14 = 270 -> but clipped slice is 263
SLICE_ROWS = HALF + 2 * NMS_HALO  # 270: chained 15-row windows need a 14-row halo


def _conv(x, w, b=None, pad=1, groups=1):
    y = jax.lax.conv_general_dilated(
        x, w, (1, 1), [(pad, pad), (pad, pad)],
        dimension_numbers=('NCHW', 'OIHW', 'NCHW'),
        feature_group_count=groups)
    if b is not None:
        y = y + b[None, :, None, None]
    return y


def _bn(x, s, b):
    return x * s[None, :, None, None] + b[None, :, None, None]


def _inv_res(x, dw_w, bna_s, bna_b, pw_w, bnb_s, bnb_b):
    h = _conv(x, dw_w, pad=1, groups=x.shape[1])
    h = jnp.clip(_bn(h, bna_s, bna_b), 0.0, 6.0)
    h = _conv(h, pw_w, pad=0)
    h = _bn(h, bnb_s, bnb_b)
    return x + h


def _pool_h_then_w(x, init, op):
    p = NMS_K // 2
    x = jax.lax.reduce_window(x, init, op, (1, 1, NMS_K, 1), (1, 1, 1, 1),
                              [(0, 0), (0, 0), (p, p), (0, 0)])
    x = jax.lax.reduce_window(x, init, op, (1, 1, 1, NMS_K), (1, 1, 1, 1),
                              [(0, 0), (0, 0), (0, 0), (p, p)])
    return x


def _shard_fn(photo, off_nms, off_out,
              w0, b0, dw1_w, bn1a_s, bn1a_b, pw1_w, bn1b_s, bn1b_b,
              dw2_w, bn2a_s, bn2a_b, pw2_w, bn2b_s, bn2b_b, ws, bs, scale_list):
    # photo: (1, 1, H, W) -- full image; conv stack + exact instance norm
    x = _conv(photo, w0, b0)
    x = _inv_res(x, dw1_w, bn1a_s, bn1a_b, pw1_w, bn1b_s, bn1b_b)
    x = _inv_res(x, dw2_w, bn2a_s, bn2a_b, pw2_w, bn2b_s, bn2b_b)
    s = _conv(x, ws, bs)                                   # (1, S, H, W)
    mu = s.mean(axis=(2, 3), keepdims=True)
    var = s.var(axis=(2, 3), keepdims=True)
    y = (s - mu) * jax.lax.rsqrt(var + 1e-5)
    y = jax.nn.leaky_relu(y, negative_slope=0.01)

    # NMS stage on own half only: rows [off_nms, off_nms + 263)
    yn = jax.lax.dynamic_slice(y, (0, 0, off_nms, 0), (1, S, SLICE_ROWS, W))
    mc = yn.max(axis=1, keepdims=True)
    m = _pool_h_then_w(mc, -jnp.inf, jax.lax.max)
    e = jnp.exp(COM_NMS * (yn - m))
    se = _pool_h_then_w(e.sum(axis=1, keepdims=True), 0.0, jax.lax.add)
    probs = e / (se + EPS)

    mx = probs.max(axis=1, keepdims=True)
    e1 = jnp.exp(COM_BETA * (probs - mx))
    p1 = e1 / (e1.sum(axis=1, keepdims=True) + EPS)
    score = (probs * p1).sum(axis=1)                       # (1, 263, W)
    scale = (scale_list[None, :, None, None] * p1).sum(axis=1)

    sc = jax.lax.dynamic_slice(score, (0, off_out, 0), (1, HALF, W))[0]
    sl = jax.lax.dynamic_slice(scale, (0, off_out, 0), (1, HALF, W))[0]
    # f16 return halves the (slow) axon device->host fetch; post-softmax
    # rounding here is ~5e-4 relative, far inside tolerance.
    return sc.astype(jnp.float16), sl.astype(jnp.float16)


_PFN = None


def _get_pfn():
    global _PFN
    if _PFN is None:
        _PFN = jax.pmap(_shard_fn)
    return _PFN


def _run_sharded(inputs):
    photos = np.asarray(inputs['photos'], np.float32)
    slabs = np.empty((8, 1, 1, H, W), np.float32)
    off_nms = np.empty((8,), np.int32)
    off_out = np.empty((8,), np.int32)
    for i in range(8):
        b, half = i // 2, i % 2
        slabs[i, 0] = photos[b]
        if half == 0:
            off_nms[i], off_out[i] = 0, 0           # rows [0,263) -> out [0,256)
        else:
            off_nms[i], off_out[i] = H - SLICE_ROWS, 2 * NMS_HALO  # [242,512) -> [256,512)

    names = ['w0', 'b0', 'dw1_w', 'bn1a_s', 'bn1a_b', 'pw1_w', 'bn1b_s',
             'bn1b_b', 'dw2_w', 'bn2a_s', 'bn2a_b', 'pw2_w', 'bn2b_s',
             'bn2b_b', 'ws', 'bs', 'scale_list']
    reps = [np.broadcast_to(np.asarray(inputs[k], np.float32),
                            (8,) + np.asarray(inputs[k]).shape) for k in names]
    sc, sl = _get_pfn()(slabs, off_nms, off_out, *reps)
    sc = np.asarray(sc).reshape(B, H, W)[..., None]
    sl = np.asarray(sl).reshape(B, H, W)[..., None]
    return sc.astype(np.float32), sl.astype(np.float32)


def _run_cpu(inputs):
    cpu = jax.devices('cpu')[0]
    fin = {k: jax.device_put(np.asarray(v), cpu) for k, v in inputs.items()}

    def full(photos, w0, b0, dw1_w, bn1a_s, bn1a_b, pw1_w, bn1b_s, bn1b_b,
             dw2_w, bn2a_s, bn2a_b, pw2_w, bn2b_s, bn2b_b, ws, bs, scale_list):
        x = _conv(photos, w0, b0)
        x = _inv_res(x, dw1_w, bn1a_s, bn1a_b, pw1_w, bn1b_s, bn1b_b)
        x = _inv_res(x, dw2_w, bn2a_s, bn2a_b, pw2_w, bn2b_s, bn2b_b)
        s = _conv(x, ws, bs)
        mu = s.mean(axis=(2, 3), keepdims=True)
        var = s.var(axis=(2, 3), keepdims=True)
        y = (s - mu) * jax.lax.rsqrt(var + 1e-5)
        y = jax.nn.leaky_relu(y, negative_slope=0.01)
        mc = y.max(axis=1, keepdims=True)
        m = _pool_h_then_w(mc, -jnp.inf, jax.lax.max)
        e = jnp.exp(COM_NMS * (y - m))
        se = _pool_h_then_w(e.sum(axis=1, keepdims=True), 0.0, jax.lax.add)
        probs = e / (se + EPS)
        mx = probs.max(axis=1, keepdims=True)
        e1 = jnp.exp(COM_BETA * (probs - mx))
        p1 = e1 / (e1.sum(axis=1, keepdims=True) + EPS)
        score = (probs * p1).sum(axis=1, keepdims=True)
        scale = (scale_list[None, :, None, None] * p1).sum(axis=1, keepdims=True)
        return score.transpose(0, 2, 3, 1), scale.transpose(0, 2, 3, 1)

    sc, sl = jax.jit(full, device=cpu)(**fin)
    return np.asarray(sc), np.asarray(sl)


def kernel(**inputs):
    try:
        return _run_sharded(inputs)
    except Exception as ex:  # fall back to exact single-device compute
        import traceback
        traceback.print_exc()
        print(f"[kernel] sharded path failed ({ex!r}); using CPU fallback",
              flush=True)
        return _run_cpu(inputs)
